# revision 9
# baseline (speedup 1.0000x reference)
"""GCN encoder kernel for Trainium2, SPMD across 8 NeuronCores.

Computes (reference semantics):
    x_ = P @ (x @ W1 + b1)
    h  = P @ (1.8 * l2norm_rows(x @ W2 + b2))
where P = D^-1/2 (A + I) D^-1/2 over the edge list (by destination).

v2 design ("seg-major partial accumulation"):
  * Phase A computes u[r] = [dinv*(x@W1+b1) | dinv*1.8*l2n(x@W2+b2)] in bf16,
    replicated on every core.  x is host-prescaled by dinv (l2norm is scale
    invariant, so the dinv factor passes through branch 2); the bias enters
    via an extra contraction row (aug channel = dinv, W row = [b1|b2]).
    u rows live in 4 per-segment DRAM tensors so phase-B gathers of segment s
    depend only on segment-s writes.
  * Destinations are bin-packed on the host into NWIN uniform windows such
    that every (window, segment) edge group fits exactly T=8 128-edge tiles.
    The segment-sum one-hot S matrices are PRE-BUILT on the host in fp8
    (exact 0/1) and streamed from DRAM - no on-device one-hot construction.
  * Phase B runs seg-major: for each segment, 26 gather calls (4 windows x
    8 tiles = 4096 edges each) rotate the 4 SWDGE queues; each window's 8
    matmuls accumulate in PSUM; a single DVE op folds the psum partial
    (scaled by dinv[dst]) into a persistent bf16 accumulator per window
    (final segment writes f32 output directly).
  * Output rows come back window-permuted; the host scatters them to node
    order.
"""
import sys

import numpy as np

try:
    import concourse.bass as bass  # noqa: F401
except ImportError:
    sys.path.insert(0, "/opt/trn_rl_repo")

from contextlib import ExitStack

from ml_dtypes import bfloat16, float8_e4m3

import concourse.bass as bass
import concourse.bacc as bacc
import concourse.tile as tile
from concourse import mybir
from concourse.bass_utils import run_bass_kernel_spmd

N_CORES = 8
N = 100000
CIN = 256
COUT = 128
C = 256
NPC = N // N_CORES          # 12500 dsts per core
SEG_REAL = 25000            # real nodes per source segment
SEG_PAD = 25600             # u rows per segment (600 pad rows)
NSEG = 4
U_ROWS = NSEG * SEG_PAD     # 102400
NRT = U_ROWS // 128         # 800 row tiles
ROWG = 512                  # phase-A row group (2 psum pairs)
G = U_ROWS // ROWG          # 200 groups (50 per segment)
T = 8                       # tiles per (window, segment)
WCW = 4                     # windows per gather call
CAP = T * 128               # 1024 edge slots per (window, segment)
SCALING = 1.8


def _urow(n):
    return n + (n // SEG_REAL) * (SEG_PAD - SEG_REAL)


def _pack_global(degmat, nwin):
    """Greedy 4-dim bin packing: all N dsts (rows of degmat [N,4]) into
    8*nwin bins with per-seg load <= CAP and size <= 128.  Assigning dsts
    to cores via bin//nwin balances the per-(core,segment) edge loads
    (self-loops are heavily seg-skewed per dst range, so a fixed dst range
    per core would not fit).  Returns (core, win, slot) or None."""
    nbins = N_CORES * nwin
    order = np.argsort(-degmat.sum(1), kind="stable")
    loads = np.zeros((nbins, NSEG), np.int32)
    sizes = np.zeros(nbins, np.int32)
    win = np.empty(N, np.int32)
    slot = np.empty(N, np.int32)
    big = np.int32(1 << 30)
    for d in order:
        v = degmat[d]
        cand = (loads + v).max(1)
        cand[(loads + v > CAP).any(1) | (sizes >= 128)] = big
        b = int(np.argmin(cand))
        if cand[b] >= big:
            return None
        win[d] = b
        slot[d] = sizes[b]
        loads[b] += v
        sizes[b] += 1
    return win // nwin, win % nwin, slot


def _prep(x, edge_index, W1, b1, W2, b2):
    x = np.asarray(x, np.float32)
    src = np.asarray(edge_index[0], np.int64)
    dst = np.asarray(edge_index[1], np.int64)

    deg = (np.bincount(dst, minlength=N) + 1).astype(np.float32)
    dinv = (1.0 / np.sqrt(deg)).astype(np.float32)

    # edges + self loops
    n_all = np.arange(N, dtype=np.int64)
    src_a = np.concatenate([src, n_all])
    dst_a = np.concatenate([dst, n_all])
    seg_a = src_a // SEG_REAL

    # ---- shared (replicated) phase-A inputs ----
    dinv_u = np.zeros(U_ROWS, np.float32)
    dinv_u[_urow(n_all)] = dinv
    xt = np.zeros((CIN, U_ROWS), bfloat16)
    xt[:, _urow(n_all)] = (x * dinv[:, None]).T.astype(bfloat16)
    xt3 = np.zeros((1, U_ROWS), bfloat16)
    xt3[0, :] = dinv_u.astype(bfloat16)
    wc = np.concatenate([W1, W2], axis=1).astype(bfloat16)        # [256, 256]
    wc3 = np.concatenate([b1, b2]).astype(bfloat16)[None, :]      # [1, 256]
    d_safe = np.where(dinv_u > 0, dinv_u, 1.0)
    dinvr1 = np.ascontiguousarray(
        (1.0 / (SCALING * d_safe)).reshape(NRT, 128).T)           # [128, NRT]

    # ---- global dst -> (core, window, slot) assignment ----
    degmat = np.zeros((N, NSEG), np.int32)
    np.add.at(degmat, (dst_a, seg_a), 1)
    nwin = 104
    pk = _pack_global(degmat, nwin)
    if pk is None:
        nwin = 112
        pk = _pack_global(degmat, nwin)
        assert pk is not None, "window packing failed"
    core, win, slot = pk
    nchunk = nwin // WCW
    ncalls = NSEG * nchunk

    core_of = core[dst_a]
    in_maps = []
    dstmaps = []
    for k in range(N_CORES):
        m = core_of == k
        src_k = src_a[m]
        dst_k = dst_a[m]
        seg_k = seg_a[m]

        w_e = win[dst_k]
        o = np.lexsort((src_k, w_e, seg_k))
        s_o, w_o, src_o, dst_o = seg_k[o], w_e[o], src_k[o], dst_k[o]
        grp = s_o * nwin + w_o                       # non-decreasing
        cnt = np.bincount(grp, minlength=NSEG * nwin)
        assert cnt.max() <= CAP
        start = np.zeros(NSEG * nwin, np.int64)
        start[1:] = np.cumsum(cnt)[:-1]
        pos = np.arange(src_o.shape[0], dtype=np.int64) - start[grp]
        call = s_o * nchunk + w_o // WCW
        p_in = (w_o % WCW) * CAP + pos               # [0, 4096)

        iv = (src_o - s_o * SEG_REAL).astype(np.int16)
        edi16 = np.zeros((ncalls, 16, (WCW * CAP) // 16), np.int16)
        edi16[call, p_in % 16, p_in // 16] = iv
        edi = np.tile(edi16, (1, 8, 1))              # [ncalls, 128, 256]

        sdat = np.zeros((ncalls, 128, WCW * T * 128), float8_e4m3)
        scol = (p_in // 128) * 128 + slot[dst_o]
        sdat[call, p_in % 128, scol] = 1.0

        dk = np.flatnonzero(core == k)
        dd = np.zeros((128, nwin), np.float32)
        dd[slot[dk], win[dk]] = dinv[dk]
        dmap = np.full(nwin * 128, -1, np.int64)
        dmap[win[dk] * 128 + slot[dk]] = dk

        in_maps.append({
            "xt": xt, "xt3": xt3, "wc": wc, "wc3": wc3,
            "dinvr1": dinvr1, "dinvd": dd, "edi": edi, "sdat": sdat,
        })
        dstmaps.append(dmap)

    d = dict(NWIN=nwin, NCHUNK=nwin // WCW, NCALLS=NSEG * (nwin // WCW))
    return in_maps, (d, dstmaps)


def _build(d):
    f32, bf16 = mybir.dt.float32, mybir.dt.bfloat16
    fp8, i16 = mybir.dt.float8e4, mybir.dt.int16
    NWIN, NCHUNK = d["NWIN"], d["NCHUNK"]
    NCALLS = d["NCALLS"]

    nc = bacc.Bacc("TRN2", target_bir_lowering=False, debug=False,
                   num_swdge_queues=4)
    xt_d = nc.dram_tensor("xt", [CIN, U_ROWS], bf16, kind="ExternalInput")
    xt3_d = nc.dram_tensor("xt3", [1, U_ROWS], bf16, kind="ExternalInput")
    wc_d = nc.dram_tensor("wc", [CIN, C], bf16, kind="ExternalInput")
    wc3_d = nc.dram_tensor("wc3", [1, C], bf16, kind="ExternalInput")
    dinvr1_d = nc.dram_tensor("dinvr1", [128, NRT], f32, kind="ExternalInput")
    dinvd_d = nc.dram_tensor("dinvd", [128, NWIN], f32, kind="ExternalInput")
    edi_d = nc.dram_tensor("edi", [NCALLS, 128, WCW * CAP // 16], i16,
                           kind="ExternalInput")
    sdat_d = nc.dram_tensor("sdat", [NCALLS, 128, WCW * T * 128], fp8,
                            kind="ExternalInput")
    out_d = nc.dram_tensor("out", [NWIN * 128, C], f32, kind="ExternalOutput")
    u_sd = [nc.dram_tensor(f"useg{s}", [SEG_PAD, C], bf16) for s in range(NSEG)]

    with ExitStack() as ctx:
        tc = ctx.enter_context(tile.TileContext(nc))
        const_p = ctx.enter_context(tc.tile_pool(name="const", bufs=1))
        xa_p = ctx.enter_context(tc.tile_pool(name="xa", bufs=3))
        sq_p = ctx.enter_context(tc.tile_pool(name="sq", bufs=4))
        col_p = ctx.enter_context(tc.tile_pool(name="col", bufs=8))
        ua_p = ctx.enter_context(tc.tile_pool(name="ua", bufs=4))
        ed_p = ctx.enter_context(tc.tile_pool(name="ed", bufs=4))
        ss_p = ctx.enter_context(tc.tile_pool(name="ss", bufs=4))
        msg_p = ctx.enter_context(tc.tile_pool(name="msg", bufs=5))
        acc_p = ctx.enter_context(tc.tile_pool(name="acc", bufs=1))
        out_p = ctx.enter_context(tc.tile_pool(name="o", bufs=3))
        psa_p = ctx.enter_context(tc.tile_pool(name="psa", bufs=3, space="PSUM"))
        psb_p = ctx.enter_context(tc.tile_pool(name="psb", bufs=5, space="PSUM"))

        # constants
        wc_t = [const_p.tile([128, C], bf16, name=f"wct{kc}", tag=f"wc{kc}")
                for kc in range(2)]
        for kc in range(2):
            nc.sync.dma_start(out=wc_t[kc][:],
                              in_=wc_d[kc * 128:(kc + 1) * 128, :])
        wc3_t = const_p.tile([1, C], bf16)
        nc.sync.dma_start(out=wc3_t[:], in_=wc3_d[:, :])
        dinvr1_t = const_p.tile([128, NRT], f32)
        nc.sync.dma_start(out=dinvr1_t[:], in_=dinvr1_d[:, :])
        dinvd_t = const_p.tile([128, NWIN], f32)
        nc.sync.dma_start(out=dinvd_t[:], in_=dinvd_d[:, :])
        eps_t = const_p.tile([128, 1], f32)
        nc.vector.memset(eps_t[:], 1e-24)

        accs = [acc_p.tile([128, C], bf16, name=f"acc{w}", tag=f"acc{w}")
                for w in range(NWIN)]

        # ---- phase A ----
        for g in range(G):
            s = g // (G // NSEG)
            lbase = (g % (G // NSEG)) * ROWG
            xg = [xa_p.tile([128, ROWG], bf16, name=f"xg{kc}", tag=f"xg{kc}")
                  for kc in range(2)]
            for kc in range(2):
                nc.sync.dma_start(
                    out=xg[kc][:],
                    in_=xt_d[kc * 128:(kc + 1) * 128,
                             g * ROWG:(g + 1) * ROWG])
            xb = xa_p.tile([1, ROWG], bf16, tag="xb")
            nc.sync.dma_start(out=xb[:], in_=xt3_d[:, g * ROWG:(g + 1) * ROWG])
            for half in range(2):
                pp = psa_p.tile([128, 2, C], f32)
                for j in range(2):
                    tj = half * 2 + j
                    for kc in range(2):
                        nc.tensor.matmul(
                            pp[:, j, :],
                            lhsT=xg[kc][:, tj * 128:(tj + 1) * 128],
                            rhs=wc_t[kc][:], start=(kc == 0), stop=False)
                    nc.tensor.matmul(
                        pp[:, j, :], lhsT=xb[:, tj * 128:(tj + 1) * 128],
                        rhs=wc3_t[:], start=False, stop=True)
                sc = col_p.tile([128, 2], f32, tag="sc")
                for j in range(2):
                    rt = g * 4 + half * 2 + j
                    t2s = sq_p.tile([128, COUT], f32, tag="t2s")
                    nc.vector.tensor_scalar(
                        out=t2s[:], in0=pp[:, j, COUT:],
                        scalar1=dinvr1_t[:, rt:rt + 1], scalar2=None,
                        op0=mybir.AluOpType.mult)
                    sq_t = sq_p.tile([128, COUT], f32, tag="sq")
                    nc.vector.scalar_tensor_tensor(
                        out=sq_t[:], in0=t2s[:], scalar=1.0, in1=t2s[:],
                        op0=mybir.AluOpType.mult, op1=mybir.AluOpType.mult,
                        accum_out=sc[:, j:j + 1])
                nrm = col_p.tile([128, 2], f32, tag="nrm")
                nc.scalar.activation(
                    out=nrm[:], in_=sc[:],
                    func=mybir.ActivationFunctionType.Sqrt,
                    bias=eps_t[:], scale=1.0)
                phi = col_p.tile([128, 2], f32, tag="phi")
                nc.vector.reciprocal(out=phi[:], in_=nrm[:])
                u_t = ua_p.tile([128, 2, C], bf16)
                nc.scalar.activation(
                    out=u_t[:, :, 0:COUT], in_=pp[:, :, 0:COUT],
                    func=mybir.ActivationFunctionType.Copy,
                    bias=0.0, scale=1.0)
                for j in range(2):
                    nc.scalar.activation(
                        out=u_t[:, j, COUT:], in_=pp[:, j, COUT:],
                        func=mybir.ActivationFunctionType.Copy,
                        bias=0.0, scale=phi[:, j:j + 1])
                for j in range(2):
                    lrow = lbase + half * 256 + j * 128
                    nc.sync.dma_start(
                        out=u_sd[s][lrow:lrow + 128, :], in_=u_t[:, j, :])

        # ---- phase B: seg-major ----
        for s in range(NSEG):
            for wcix in range(NCHUNK):
                c = s * NCHUNK + wcix
                ei = ed_p.tile([128, WCW * CAP // 16], i16, tag="ei")
                nc.sync.dma_start(out=ei[:], in_=edi_d[c, :, :])
                st = ss_p.tile([128, WCW * T, 128], fp8, tag="st")
                nc.sync.dma_start(
                    out=st[:],
                    in_=sdat_d[c, :, :].rearrange("p (t q) -> p t q", q=128))
                mt = msg_p.tile([128, WCW * T, C], bf16)
                nc.gpsimd.dma_gather(
                    out_ap=mt[:],
                    in_ap=u_sd[s][:, :],
                    idxs_ap=ei[:, :],
                    num_idxs=WCW * CAP,
                    num_idxs_reg=WCW * CAP,
                    elem_size=C,
                    single_packet=False,
                    queue_num=c % 4)
                for wl in range(WCW):
                    w = wcix * WCW + wl
                    ps = psb_p.tile([128, C], f32)
                    for t in range(T):
                        kk = wl * T + t
                        nc.tensor.matmul(
                            ps[:], lhsT=st[:, kk, :], rhs=mt[:, kk, :],
                            start=(t == 0), stop=(t == T - 1))
                    dcol = dinvd_t[:, w:w + 1]
                    if s == 0:
                        nc.vector.tensor_scalar(
                            out=accs[w][:], in0=ps[:], scalar1=dcol,
                            scalar2=None, op0=mybir.AluOpType.mult)
                    elif s < NSEG - 1:
                        nc.vector.scalar_tensor_tensor(
                            out=accs[w][:], in0=ps[:], scalar=dcol,
                            in1=accs[w][:], op0=mybir.AluOpType.mult,
                            op1=mybir.AluOpType.add)
                    else:
                        o_t = out_p.tile([128, C], f32)
                        nc.vector.scalar_tensor_tensor(
                            out=o_t[:], in0=ps[:], scalar=dcol,
                            in1=accs[w][:], op0=mybir.AluOpType.mult,
                            op1=mybir.AluOpType.add)
                        nc.sync.dma_start(
                            out=out_d[w * 128:(w + 1) * 128, :], in_=o_t[:])

    nc.compile()
    return nc


def _run(in_maps, meta, trace=False):
    d, dstmaps = meta
    nc = _build(d)
    res = run_bass_kernel_spmd(
        nc, in_maps, core_ids=list(range(N_CORES)), trace=trace)
    x_ = np.empty((N, COUT), np.float32)
    h = np.empty((N, COUT), np.float32)
    for k in range(N_CORES):
        outs = res.results[k]["out"]
        dmap = dstmaps[k]
        valid = dmap >= 0
        x_[dmap[valid]] = outs[valid, :COUT]
        h[dmap[valid]] = outs[valid, COUT:]
    return (h, x_), res


def kernel(x, edge_index, W1, b1, W2, b2):
    in_maps, meta = _prep(x, edge_index, W1, b1, W2, b2)
    (h, x_), _ = _run(in_maps, meta, trace=False)
    return (h, x_)
